# revision 1
# baseline (speedup 1.0000x reference)
"""DGCNN forward kernel for 8 Trainium2 NeuronCores.

Contract: kernel(**inputs) takes the FULL inputs of the reference
(x:(4,3,8192), w1..w5) and returns the FULL output (4,512,8192) fp32.

Sharding: data-parallel over batch B=4 x query-halves -> 8 cores.
Core c = 2*b + h computes queries [h*4096,(h+1)*4096) of batch item b
against all 8192 candidates of batch item b. No cross-core comm.

Per-core pipeline (query tiles of 128):
  PE    : score matmuls s_qj = 2*x_q.x_j - |x_j|^2   (fp32, K=4)
  ACT   : PSUM->SBUF copies of the (128,8192) score block
  DVE   : max8 (top-8 values) + max_index (top-8 indices) -> exact top-5
  SWDGE : indirect DMA gather of neighbor coords (128B padded rows)
  PE    : per-k fp32 transpose of [nbr;ctr] + conv1..conv5 (fp32)
  ACT   : relu epilogues
  DVE   : max-pool over K=5
  DMA   : output store
"""

import sys

if '/opt/trn_rl_repo' not in sys.path:
    sys.path.insert(0, '/opt/trn_rl_repo')

import numpy as np

import concourse.bass as bass
import concourse.tile as tile
from concourse import bacc, mybir
from concourse.bass_utils import run_bass_kernel_spmd

F32 = mybir.dt.float32
F32R = mybir.dt.float32r
U32 = mybir.dt.uint32
AF = mybir.ActivationFunctionType
ALU = mybir.AluOpType

B = 4
N = 8192          # points per batch element (candidates)
NQ = 4096         # queries per core
P = 128           # queries per tile
SG = 4            # tiles per supergroup (conv5 free dim = SG*128 = 512)
KNN = 5


def _build_program(n=N, nq=NQ, sgsz=SG, num_devices=8, stop_after=None):
    NT_ = nq // P
    NSG_ = NT_ // sgsz
    nc = bacc.Bacc("TRN2", target_bir_lowering=False, debug=False,
                   num_devices=num_devices)

    d_xt32 = nc.dram_tensor("xt32", [n, 32], F32, kind="ExternalInput").ap()
    d_srhs = nc.dram_tensor("srhs", [4, n], F32, kind="ExternalInput").ap()
    d_xq4 = nc.dram_tensor("xq4", [4, nq], F32, kind="ExternalInput").ap()
    d_w1t = nc.dram_tensor("w1t", [6, 64], F32, kind="ExternalInput").ap()
    d_w2t = nc.dram_tensor("w2t", [64, 64], F32, kind="ExternalInput").ap()
    d_w3t = nc.dram_tensor("w3t", [128, 128], F32, kind="ExternalInput").ap()
    d_w4t = nc.dram_tensor("w4t", [128, 256], F32, kind="ExternalInput").ap()
    d_w5r = nc.dram_tensor("w5r", [128, 2048], F32, kind="ExternalInput").ap()
    d_idn = nc.dram_tensor("idn", [128, 128], F32, kind="ExternalInput").ap()
    d_out = nc.dram_tensor("out", [512, nq], F32, kind="ExternalOutput").ap()

    with tile.TileContext(nc) as tc:
        with tc.tile_pool(name="consts", bufs=1) as consts, \
             tc.tile_pool(name="scores", bufs=2) as scores_pool, \
             tc.tile_pool(name="small", bufs=2) as small, \
             tc.tile_pool(name="acts", bufs=2) as acts, \
             tc.tile_pool(name="cats", bufs=2) as cats, \
             tc.tile_pool(name="ps_score", bufs=2, space="PSUM") as ps_score, \
             tc.tile_pool(name="ps_work", bufs=2, space="PSUM") as ps_work:

            srhs = consts.tile([4, n], F32)
            nc.sync.dma_start(srhs[:], d_srhs[:])
            xq4 = consts.tile([4, nq], F32)
            nc.sync.dma_start(xq4[:], d_xq4[:])
            w1t = consts.tile([6, 64], F32)
            nc.sync.dma_start(w1t[:], d_w1t[:])
            w2t = consts.tile([64, 64], F32)
            nc.sync.dma_start(w2t[:], d_w2t[:])
            w3t = consts.tile([128, 128], F32)
            nc.sync.dma_start(w3t[:], d_w3t[:])
            w4t = consts.tile([128, 256], F32)
            nc.sync.dma_start(w4t[:], d_w4t[:])
            w5r = consts.tile([128, 2048], F32)
            nc.sync.dma_start(w5r[:], d_w5r[:])
            idn = consts.tile([128, 128], F32)
            nc.sync.dma_start(idn[:], d_idn[:])
            w3r = consts.tile([128, 128], F32R)
            nc.vector.tensor_copy(w3r[:], w3t[:])
            w4r = consts.tile([128, 256], F32R)
            nc.vector.tensor_copy(w4r[:], w4t[:])
            w5rr = consts.tile([128, 2048], F32R)
            nc.vector.tensor_copy(w5rr[:], w5r[:])

            out_view = d_out.rearrange("(o p) q -> p o q", o=4)
            _early = ("scores", "topk", "gather", "ti", "conv1")

            for sg in range(NSG_):
                if stop_after in _early:
                    cat12 = cat3 = cat4a = cat4b = None
                else:
                    cat12 = cats.tile([128, sgsz * P], F32R, tag="cat12")
                    cat3 = cats.tile([128, sgsz * P], F32R, tag="cat3")
                    cat4a = cats.tile([128, sgsz * P], F32R, tag="cat4a")
                    cat4b = cats.tile([128, sgsz * P], F32R, tag="cat4b")

                for ti in range(sgsz):
                    t = sg * sgsz + ti
                    q0 = t * P

                    # ---- scores: s (128 q, n cand) ----
                    sc = scores_pool.tile([P, n], F32, tag="sc")
                    lhsq = xq4[:, q0:q0 + P]
                    for cc in range(n // 1024):
                        psc = ps_score.tile([P, 1024], F32, tag="psc")
                        c0 = cc * 1024
                        nc.tensor.matmul(psc[:, 0:512], lhsT=lhsq,
                                         rhs=srhs[:, c0:c0 + 512],
                                         start=True, stop=True)
                        nc.tensor.matmul(psc[:, 512:1024], lhsT=lhsq,
                                         rhs=srhs[:, c0 + 512:c0 + 1024],
                                         start=True, stop=True)
                        nc.scalar.activation(sc[:, c0:c0 + 1024], psc[:], AF.Copy)

                    # ---- top-5 (exact, fp32) ----
                    m8 = small.tile([P, 8], F32, tag="m8")
                    nc.vector.max(out=m8[:], in_=sc[:])
                    i8 = small.tile([P, 8], U32, tag="i8")
                    nc.vector.max_index(out=i8[:], in_max=m8[:], in_values=sc[:])

                    if stop_after == "scores":
                        dbg = small.tile([P, P], F32, tag="dbg")
                        nc.vector.tensor_copy(dbg[:], sc[:, 0:P])
                        nc.sync.dma_start(d_out[0:P, t * P:(t + 1) * P], dbg[:])
                        continue
                    if stop_after == "topk":
                        dbg = small.tile([P, P], F32, tag="dbg")
                        nc.vector.memset(dbg[:], 0.0)
                        nc.vector.tensor_copy(dbg[:, 0:8], m8[:])
                        nc.vector.tensor_copy(dbg[:, 8:16], i8[:])
                        nc.sync.dma_start(d_out[0:P, t * P:(t + 1) * P], dbg[:])
                        continue

                    # ---- gather neighbor coords: g[q, k, :] = xt32[idx[q,k]] ----
                    # one offset per partition per DMA (multi-offset indirect
                    # DMA scrambles on HW)
                    g = small.tile([P, KNN, 32], F32, tag="g")
                    for k in range(KNN):
                        nc.gpsimd.indirect_dma_start(
                            out=g[:, k, :],
                            out_offset=None,
                            in_=d_xt32[:],
                            in_offset=bass.IndirectOffsetOnAxis(
                                ap=i8[:, k:k + 1], axis=0),
                        )

                    if stop_after == "gather":
                        dbg = small.tile([P, P], F32, tag="dbg")
                        nc.vector.tensor_copy(
                            dbg[:], g[:].rearrange("p k j -> p (k j)")[:, 0:P])
                        nc.sync.dma_start(d_out[0:P, t * P:(t + 1) * P], dbg[:])
                        continue

                    # ---- assemble TI[q, k, 0:6] = [nbr_k(3), ctr(3)] ----
                    # ctr = gathered top-1 row (self) broadcast over k.
                    tin = small.tile([P, KNN, 6], F32, tag="tin")
                    nc.vector.tensor_copy(tin[:, :, 0:3], g[:, :, 0:3])
                    nc.vector.tensor_copy(
                        tin[:, :, 3:6], g[:, 0:1, 0:3].to_broadcast([P, KNN, 3]))

                    if stop_after == "ti":
                        dbg = small.tile([P, P], F32, tag="dbg")
                        nc.vector.memset(dbg[:], 0.0)
                        nc.vector.tensor_copy(
                            dbg[:, 0:30], tin[:].rearrange("p k j -> p (k j)"))
                        nc.sync.dma_start(d_out[0:P, t * P:(t + 1) * P], dbg[:])
                        continue

                    # ---- per-k transpose (128,6)->(6,128), conv1 K=6 ----
                    ps_tp = ps_work.tile([8, KNN * P], F32, tag="work")
                    for k in range(KNN):
                        nc.tensor.transpose(ps_tp[0:6, k * P:(k + 1) * P],
                                            tin[:, k, :], idn[:])
                    tps = small.tile([8, KNN * P], F32, tag="tps")
                    nc.scalar.activation(tps[0:6, :], ps_tp[0:6, :], AF.Copy)

                    ps_h1 = ps_work.tile([64, KNN * P], F32, tag="work")
                    for k in range(KNN):
                        nc.tensor.matmul(ps_h1[:, k * P:(k + 1) * P],
                                         lhsT=w1t[:],
                                         rhs=tps[0:6, k * P:(k + 1) * P],
                                         start=True, stop=True)
                    h12 = acts.tile([128, KNN, P], F32R, tag="h12")
                    h1 = h12[0:64]
                    nc.scalar.activation(
                        h12[:].rearrange("c k q -> c (k q)")[0:64, :],
                        ps_h1[:], AF.Relu)

                    if stop_after == "conv1":
                        dbg = small.tile([P, P], F32, tag="dbg")
                        nc.vector.memset(dbg[:], 0.0)
                        nc.vector.tensor_copy(
                            dbg[0:64, :],
                            h12[0:64].rearrange("c k q -> c (k q)")[:, 0:P].bitcast(F32))
                        nc.sync.dma_start(d_out[0:P, t * P:(t + 1) * P], dbg[:])
                        continue

                    # ---- conv2 (output placed at PSUM partitions 64:128) ----
                    ps_c2 = ps_work.tile([128, KNN * P], F32, tag="work")
                    h1f = h12[0:64].rearrange("c k q -> c (k q)").bitcast(F32)
                    nc.tensor.matmul(ps_c2[64:128, 0:512], lhsT=w2t[:],
                                     rhs=h1f[:, 0:512], start=True, stop=True)
                    nc.tensor.matmul(ps_c2[64:128, 512:640], lhsT=w2t[:],
                                     rhs=h1f[:, 512:640], start=True, stop=True)
                    nc.scalar.activation(
                        h12[:].rearrange("c k q -> c (k q)")[64:128, :],
                        ps_c2[64:128, :], AF.Relu)
                    h2 = h12

                    # ---- conv3 (weights live at partitions 64:128) ----
                    ps_c3 = ps_work.tile([128, KNN * P], F32, tag="work")
                    h2f = h2[:].rearrange("c k q -> c (k q)")
                    nc.tensor.matmul(ps_c3[:, 0:512], lhsT=w3r[64:128, :],
                                     rhs=h2f[64:128, 0:512], start=True, stop=True)
                    nc.tensor.matmul(ps_c3[:, 512:640], lhsT=w3r[64:128, :],
                                     rhs=h2f[64:128, 512:640], start=True, stop=True)
                    h3 = acts.tile([128, KNN, P], F32R, tag="h3")
                    nc.scalar.activation(h3[:].rearrange("c k q -> c (k q)"),
                                         ps_c3[:], AF.Relu)

                    # ---- conv4 (256 out channels = two 128 halves) ----
                    h3f = h3[:].rearrange("c k q -> c (k q)")
                    h4 = []
                    for half in range(2):
                        ps_c4 = ps_work.tile([128, KNN * P], F32, tag="work")
                        w4sl = w4r[:, half * 128:(half + 1) * 128]
                        nc.tensor.matmul(ps_c4[:, 0:512], lhsT=w4sl,
                                         rhs=h3f[:, 0:512], start=True, stop=True)
                        nc.tensor.matmul(ps_c4[:, 512:640], lhsT=w4sl,
                                         rhs=h3f[:, 512:640], start=True, stop=True)
                        h4t = acts.tile([128, KNN, P], F32R, tag=f"h4{half}")
                        nc.scalar.activation(h4t[:].rearrange("c k q -> c (k q)"),
                                             ps_c4[:], AF.Relu)
                        h4.append(h4t)

                    # ---- max over K=5 into the supergroup cat tiles ----
                    csl = slice(ti * P, (ti + 1) * P)
                    nc.vector.tensor_reduce(cat12[:, csl],
                                            h12[:].rearrange("c k q -> c q k"),
                                            axis=mybir.AxisListType.X, op=ALU.max)
                    nc.vector.tensor_reduce(cat3[:, csl],
                                            h3[:].rearrange("c k q -> c q k"),
                                            axis=mybir.AxisListType.X, op=ALU.max)
                    nc.vector.tensor_reduce(cat4a[:, csl],
                                            h4[0][:].rearrange("c k q -> c q k"),
                                            axis=mybir.AxisListType.X, op=ALU.max)
                    nc.vector.tensor_reduce(cat4b[:, csl],
                                            h4[1][:].rearrange("c k q -> c q k"),
                                            axis=mybir.AxisListType.X, op=ALU.max)

                if stop_after in _early:
                    continue
                if stop_after == "pools":
                    dbg2 = small.tile([P, sgsz * P], F32, tag="dbg2")
                    nc.vector.tensor_copy(dbg2[:], cat3[:])
                    nc.sync.dma_start(
                        d_out[0:P, sg * sgsz * P:(sg + 1) * sgsz * P], dbg2[:])
                    continue

                # ---- conv5 over the supergroup: K=512 as 4 chunks of 128 ----
                kchunk_rhs = (cat12, cat3, cat4a, cat4b)
                for o in range(4):
                    ps_c5 = ps_work.tile([128, sgsz * P], F32, tag="work")
                    for kk in range(4):
                        nc.tensor.matmul(
                            ps_c5[:],
                            lhsT=w5rr[:, kk * 512 + o * 128:kk * 512 + (o + 1) * 128],
                            rhs=kchunk_rhs[kk][:],
                            start=(kk == 0), stop=(kk == 3))
                    ostage = small.tile([128, sgsz * P], F32, tag="ostage")
                    nc.scalar.activation(ostage[:], ps_c5[:], AF.Relu)
                    nc.sync.dma_start(
                        out_view[:, o, sg * sgsz * P:(sg + 1) * sgsz * P],
                        ostage[:])

    nc.compile()
    return nc


_PROGRAM = None


def _get_program():
    global _PROGRAM
    if _PROGRAM is None:
        _PROGRAM = _build_program()
    return _PROGRAM


def _host_inputs(xb, h, w1, w2, w3, w4, w5, n=N, nq=NQ):
    """Per-core input map for batch element xb (3,n), query slice h."""
    xb = np.ascontiguousarray(xb, dtype=np.float32)
    sq = (xb * xb).sum(axis=0, dtype=np.float32)

    xt32 = np.zeros((n, 32), np.float32)
    xt32[:, 0:3] = xb.T

    srhs = np.empty((4, n), np.float32)
    srhs[0:3] = 2.0 * xb
    srhs[3] = -sq

    q = slice(h * nq, (h + 1) * nq)
    xq4 = np.empty((4, nq), np.float32)
    xq4[0:3] = xb[:, q]
    xq4[3] = 1.0

    w3t = np.zeros((128, 128), np.float32)
    w3t[64:128, :] = w3.T

    w5t = w5.T.astype(np.float32)  # (512 in, 512 out)
    w5r = np.zeros((128, 2048), np.float32)
    for kk in range(4):
        for o in range(4):
            w5r[:, kk * 512 + o * 128:kk * 512 + (o + 1) * 128] = \
                w5t[kk * 128:(kk + 1) * 128, o * 128:(o + 1) * 128]

    return {
        "xt32": xt32,
        "srhs": srhs,
        "xq4": xq4,
        "w1t": np.ascontiguousarray(w1.T, np.float32),
        "w2t": np.ascontiguousarray(w2.T, np.float32),
        "w3t": w3t,
        "w4t": np.ascontiguousarray(w4.T, np.float32),
        "w5r": w5r,
        "idn": np.eye(128, dtype=np.float32),
    }


def kernel(x, w1, w2, w3, w4, w5, _trace=False, _trace_kwargs=None):
    x = np.asarray(x, np.float32)
    w1 = np.asarray(w1, np.float32)
    w2 = np.asarray(w2, np.float32)
    w3 = np.asarray(w3, np.float32)
    w4 = np.asarray(w4, np.float32)
    w5 = np.asarray(w5, np.float32)
    assert x.shape == (B, 3, N), x.shape

    nc = _get_program()
    in_maps = []
    for b in range(B):
        for h in range(2):
            in_maps.append(_host_inputs(x[b], h, w1, w2, w3, w4, w5))

    kw = {}
    if _trace:
        kw = dict(trace=True, **(_trace_kwargs or {}))
    res = run_bass_kernel_spmd(nc, in_maps, list(range(8)), **kw)

    out = np.empty((B, 512, N), np.float32)
    for b in range(B):
        out[b, :, 0:NQ] = res.results[2 * b]["out"]
        out[b, :, NQ:N] = res.results[2 * b + 1]["out"]
    if _trace:
        return out, res
    return out

